# revision 69
# baseline (speedup 1.0000x reference)
"""HarmonicNoiseSynth Trainium2 kernel (v2).

Sharding: 8 cores = 4 batches x 2 harmonic halves (64 harmonics each); each
core also handles 16 of the 32 noise bands; every core runs the modulator
path on its first 4 local harmonics but the host only consumes it from j==0
cores (where those are the true modulators, harmonics 0..3).

Wire format (host-quantized to cut the ~30 MB/s axon transfer 3x):
  freq  -> uint16 phase units q = round(f * 65536/48000); phase quantization
           errors are independent per sample so the phase error random-walks:
           sigma ~ (2pi/65536)*sqrt(T)*0.29 ~ 5e-3 rad. Negligible.
  amp   -> uint8 (x255), dequant via SWDGE DMA cast to bf16 + 1/256 in lhsT,
           with a 256/255 host-side correction.
  nba   -> uint8 (x255), bands -> bf16.

Device pipeline per [128, 4096] half-slice (partition p = h_local*8 + tb,
tb = time slice of 8192; free dim = time within slice):
  1. scan (DVE): cumsum of q in fp32 (exact: chunk sums < 2^24), int32 out,
     initial = host-computed (phi0 + carry) mod 65536 per 1024-chunk.
  2. AND 0xFFFF (DVE): phase mod 65536 (per-element range reduction).
  3. Sin (ACT): cos via Sin(2pi/65536 * m - pi) = -cos(theta); the -1 is
     folded into lhsT. bf16 out.
  4. amp mul (POOL, bf16); 5. PE matmul partial sums, accumulated across the
     4 harmonic groups in PSUM ([64, 4096] tile: rows 0-7/8-15 hc/nz half 0,
     rows 32-39/40-47 hc/nz half 1).
The modulator path runs at the end, reusing the PSUM tile after evacuation;
|arcsin(0.99 c)|^e = Exp(e * Ln((2/pi)*Arctan(y*Rsqrt(1-y^2)))), y=Abs(.99c)
with the abs/scale/exponent multiplies folded into ACT affine slots.
"""
import sys

sys.path.insert(0, "/opt/trn_rl_repo")

import numpy as np
import ml_dtypes

import concourse.bass as bass
import concourse.mybir as mybir
from concourse.tile import TileContext
from concourse.bass_utils import run_bass_kernel_spmd

F = mybir.dt.float32
BF = mybir.dt.bfloat16
U8 = mybir.dt.uint8
U16 = mybir.dt.uint16
I32 = mybir.dt.int32

SR = 48000.0
B, H, NB, T = 4, 128, 32, 65536
NTB = 8          # time slices on partitions
TS = T // NTB    # 8192 per slice
TC = 1024        # scan chunk columns (cumsum stays < 2^24: exact fp32)
HB = 4096        # half-slice processed per pipeline step
NG = 4           # h-groups of 16 harmonics
HG = 16
# 10-bit phase units: freq ships as uint8 *increments of the rounded
# cumulative phase* (error-feedback quantization: q[t] = round(S[t]) -
# round(S[t-1]) with S = cumsum(f*1024/48000)), so the accumulated phase
# error is bounded by half a quantum (pi/1024 rad) at every t instead of
# random-walking. Increments are <= 173 -> uint8.
MODQ = 1024.0
SCALE_Q = float(MODQ / SR)
KP = float(2.0 * np.pi / MODQ)

_CACHE = {}

# which (h5, g) half-slices run the amp*cos multiply on DVE (rest on Pool)
# and where the noise mul runs: tuned with the TimelineSim cost model.
MUL_DVE_STEPS = {0, 1, 2, 3, 4, 5, 6, 7}
NOISE_MUL_DVE = False


def _split_multiwaits(nc):
    """This walrus build supports ONE sync wait per instruction; hoist extras
    onto single-wait NoOps inserted before the offending instruction."""
    ctr = 0
    for f in nc.m.functions:
        for bb in f.blocks:
            insts = list(bb.instructions)
            if not any(i.sync_info is not None and len(i.sync_info.on_wait) > 1
                       for i in insts):
                continue
            new = []
            for inst in insts:
                si = inst.sync_info
                if si is not None and len(si.on_wait) > 1:
                    waits = list(si.on_wait)
                    for w in waits[:-1]:
                        ctr += 1
                        nop = mybir.InstNoOp(name=f"mwsplit_{ctr}",
                                             engine=inst.engine)
                        nop.sync_info = mybir.SyncInfo(on_wait=[w], on_update=[])
                        new.append(nop)
                    inst.sync_info = mybir.SyncInfo(on_wait=[waits[-1]],
                                                    on_update=list(si.on_update))
                new.append(inst)
            bb.instructions = new
    return ctr


def _build():
    nc = bass.Bass("TRN2")
    HN = H // 2  # 64 harmonics per core

    q_ds = [nc.dram_tensor(f"q{k}", [8, T], U8, kind="ExternalInput")
            for k in range(8)]
    amp_ds = [nc.dram_tensor(f"amp{k}", [8, T], U8, kind="ExternalInput")
              for k in range(8)]
    nba_ds = [nc.dram_tensor(f"nba{k}", [4, T], U8, kind="ExternalInput")
              for k in range(4)]
    nbb_ds = [nc.dram_tensor(f"nbb{k}", [4, T], BF, kind="ExternalInput")
              for k in range(4)]
    init_d = nc.dram_tensor("init", [128, 32], F, kind="ExternalInput")
    lhsT_d = nc.dram_tensor("lhsT", [128, 64], BF, kind="ExternalInput")
    wlhsT_d = nc.dram_tensor("wlhsT", [128, 32], BF, kind="ExternalInput")
    ecol_d = nc.dram_tensor("ecol", [128, 1], F, kind="ExternalInput")

    out_d = nc.dram_tensor("out", [16, TS], BF, kind="ExternalOutput")
    md_d = nc.dram_tensor("md_out", [2, 32, TC], BF, kind="ExternalOutput")

    # [64, 8192] each: tensor k covers harmonics 8k..8k+8 -> partitions
    # (h_local*8 + tb) within its 64-row slab
    q_rs = [d[:, :].rearrange("h (tb t) -> (h tb) t", tb=NTB) for d in q_ds]
    amp_rs = [d[:, :].rearrange("h (tb t) -> (h tb) t", tb=NTB)
              for d in amp_ds]                                     # [64, 8192]
    nba_rs = [d[:, :].rearrange("n (tb t) -> (n tb) t", tb=NTB)
              for d in nba_ds]                                     # [32, 8192]
    nbb_rs = [d[:, :].rearrange("n (tb t) -> (n tb) t", tb=NTB)
              for d in nbb_ds]                                     # [32, 8192]

    with TileContext(nc) as tc:
        with tc.tile_pool(name="sm", bufs=1) as sm, \
             tc.tile_pool(name="st", bufs=2) as st, \
             tc.tile_pool(name="pp", bufs=1, space="PSUM") as pp:

            lhsT = sm.tile([128, 64], BF)
            nc.sync.dma_start(out=lhsT, in_=lhsT_d[:, :])
            wlhsT = sm.tile([128, 32], BF)
            nc.sync.dma_start(out=wlhsT, in_=wlhsT_d[:, :])
            ecol = sm.tile([128, 1], F)
            nc.sync.dma_start(out=ecol, in_=ecol_d[:, :])
            init_sb = sm.tile([128, 32], F)
            nc.sync.dma_start(out=init_sb, in_=init_d[:, :])
            bsin = sm.tile([128, 1], F)
            nc.vector.memset(bsin, -np.pi)
            bone = sm.tile([128, 1], F)
            nc.vector.memset(bone, 1.0)
            stg = [sm.tile([128, TC], BF, tag=f"stg{i}", name=f"stg{i}")
                   for i in range(2)]
            # rows 0-7 hold hc, rows 32-39 hold nz (partition-aligned w/ psum)
            hcnz = [sm.tile([40, HB], BF, tag=f"hcnz{i}", name=f"hcnz{i}")
                    for i in range(2)]

            # PSUM (matmul out base partition must be 0/32/64): rows 0-7 hc,
            # rows 32-39 nz for the current half; halves run sequentially.
            ps = pp.tile([64, HB], F, name="ps")

            for h5 in range(2):
                cols = slice(h5 * HB, (h5 + 1) * HB)
                for g in range(NG):
                    col0 = g * 8 + h5 * 4
                    qt = st.tile([128, HB], U8, tag="qt")
                    for k in range(2):
                        nc.sync.dma_start(
                            out=qt[k * 64:(k + 1) * 64, :],
                            in_=q_rs[g * 2 + k][:, cols])
                    at = st.tile([128, HB], BF, tag="at")
                    for k in range(2):
                        nc.gpsimd.dma_start(
                            out=at[k * 64:(k + 1) * 64, :],
                            in_=amp_rs[g * 2 + k][:, cols])
                    y = st.tile([128, HB], I32, tag="y")
                    for c in range(HB // TC):
                        nc.vector.tensor_tensor_scan(
                            out=y[:, c * TC:(c + 1) * TC],
                            data0=qt[:, c * TC:(c + 1) * TC],
                            data1=qt[:, c * TC:(c + 1) * TC],
                            initial=init_sb[:, col0 + c:col0 + c + 1],
                            op0=mybir.AluOpType.add,
                            op1=mybir.AluOpType.bypass)
                    nc.vector.tensor_scalar(out=y, in0=y, scalar1=0x3FF,
                                            scalar2=None,
                                            op0=mybir.AluOpType.bitwise_and)
                    cosb = st.tile([128, HB], BF, tag="cosb")
                    nc.scalar.activation(out=cosb, in_=y,
                                         func=mybir.ActivationFunctionType.Sin,
                                         scale=KP, bias=bsin)
                    if g == 0:
                        for cl in range(4):
                            nc.sync.dma_start(
                                out=stg[h5][cl * 32:(cl + 1) * 32, :],
                                in_=cosb[0:32, cl * TC:(cl + 1) * TC])
                    if h5 * 4 + g in MUL_DVE_STEPS:
                        nc.vector.tensor_mul(out=cosb, in0=cosb, in1=at)
                    else:
                        nc.gpsimd.tensor_mul(out=cosb, in0=cosb, in1=at)
                    for s in range(HB // 512):
                        nc.tensor.matmul(
                            ps[0:8, s * 512:(s + 1) * 512],
                            lhsT[:, 0:8],
                            cosb[:, s * 512:(s + 1) * 512],
                            start=(g == 0), stop=(g == NG - 1))

                # noise: 16 bands x 8 tb on partitions
                bt = st.tile([128, HB], BF, tag="bt", bufs=1)
                for k in range(4):
                    nc.sync.dma_start(out=bt[k * 32:(k + 1) * 32, :],
                                      in_=nbb_rs[k][:, cols])
                an = st.tile([128, HB], BF, tag="an", bufs=1)
                for k in range(4):
                    nc.gpsimd.dma_start(out=an[k * 32:(k + 1) * 32, :],
                                        in_=nba_rs[k][:, cols])
                if NOISE_MUL_DVE:
                    nc.vector.tensor_mul(out=bt, in0=bt, in1=an)
                else:
                    nc.gpsimd.tensor_mul(out=bt, in0=bt, in1=an)
                for s in range(HB // 512):
                    nc.tensor.matmul(ps[32:40, s * 512:(s + 1) * 512],
                                     lhsT[:, 32:40],
                                     bt[:, s * 512:(s + 1) * 512],
                                     start=True, stop=True)

                # evacuate hc (psum rows 0-7) + nz (rows 32-39)
                nc.scalar.copy(out=hcnz[h5][0:8, :], in_=ps[0:8, :])
                nc.scalar.copy(out=hcnz[h5][32:40, :], in_=ps[32:40, :])
                nc.sync.dma_start(out=out_d[0:8, h5 * HB:(h5 + 1) * HB],
                                  in_=hcnz[h5][0:8, :])
                nc.sync.dma_start(out=out_d[8:16, h5 * HB:(h5 + 1) * HB],
                                  in_=hcnz[h5][32:40, :])

            # ---- modulator path (staged -cos of local harmonics 0..3) ----
            ys, y2s = [], []
            for h5 in range(2):
                yv = sm.tile([128, TC], F, tag=f"my{h5}", name=f"my{h5}")
                nc.scalar.activation(out=yv, in_=stg[h5],
                                     func=mybir.ActivationFunctionType.Abs,
                                     scale=0.99)
                ys.append(yv)
            for h5 in range(2):
                y2 = sm.tile([128, TC], F, tag=f"my2{h5}", name=f"my2{h5}")
                nc.vector.tensor_mul(out=y2, in0=ys[h5], in1=ys[h5])
                y2s.append(y2)
            for h5 in range(2):   # s = sqrt(1 - y^2)
                nc.scalar.activation(out=y2s[h5], in_=y2s[h5],
                                     func=mybir.ActivationFunctionType.Sqrt,
                                     scale=-1.0, bias=bone)
            for h5 in range(2):   # r = 1/s
                nc.vector.reciprocal(out=y2s[h5], in_=y2s[h5])
            for h5 in range(2):   # t = y * r  (= tan(arcsin y))
                nc.vector.tensor_mul(out=ys[h5], in0=ys[h5], in1=y2s[h5])
            for h5 in range(2):
                nc.scalar.activation(out=ys[h5], in_=ys[h5],
                                     func=mybir.ActivationFunctionType.Arctan)
            for h5 in range(2):   # l = ln((2/pi) * arctan)
                nc.scalar.activation(out=ys[h5], in_=ys[h5],
                                     func=mybir.ActivationFunctionType.Ln,
                                     scale=float(2.0 / np.pi))
            shp = []
            for h5 in range(2):   # shaped = exp(e * l), bf16 for the matmul
                sb = sm.tile([128, TC], BF, tag=f"msh{h5}", name=f"msh{h5}")
                nc.scalar.activation(out=sb, in_=ys[h5],
                                     func=mybir.ActivationFunctionType.Exp,
                                     scale=ecol)
                shp.append(sb)
            for h5 in range(2):   # reuse evacuated psum banks for md
                mps = ps[0:32, h5 * TC:(h5 + 1) * TC]
                for s in range(TC // 512):
                    nc.tensor.matmul(mps[:, s * 512:(s + 1) * 512], wlhsT,
                                     shp[h5][:, s * 512:(s + 1) * 512],
                                     start=True, stop=True)
                mcp = sm.tile([32, TC], BF, tag=f"mcp{h5}", name=f"mcp{h5}")
                nc.scalar.copy(out=mcp, in_=mps)
                nc.sync.dma_start(out=md_d[h5, :, :], in_=mcp)

    _split_multiwaits(nc)
    return nc


def _run_cores(nc, in_maps):
    """First call: canonical run_bass_kernel_spmd (compiles the NEFF via the
    neuronx hook). Later calls: a cached jit of the same bass2jax executable —
    rebuilding the jit per call re-traces and re-lowers the Bass module each
    time, which costs seconds."""
    if "exec" not in _CACHE:
        res = run_bass_kernel_spmd(nc, in_maps, core_ids=list(range(8)))
        import jax
        import concourse.bass2jax as b2j
        import concourse.mybir as mb
        from jax.sharding import Mesh, PartitionSpec
        from jax.experimental.shard_map import shard_map

        pname = (nc.partition_id_tensor.name if nc.partition_id_tensor
                 else None)
        in_names, out_names, out_avals, zero_shapes = [], [], [], []
        for alloc in nc.m.functions[0].allocations:
            if not isinstance(alloc, mb.MemoryLocationSet):
                continue
            name = alloc.memorylocations[0].name
            if alloc.kind == "ExternalInput":
                if name != pname:
                    in_names.append(name)
            elif alloc.kind == "ExternalOutput":
                out_names.append(name)
                shape = tuple(alloc.tensor_shape)
                dtype = mb.dt.np(alloc.dtype)
                out_avals.append(jax.core.ShapedArray(shape, dtype))
                zero_shapes.append((shape, dtype))
        n_params = len(in_names)
        all_names = in_names + out_names
        if pname is not None:
            all_names = all_names + [pname]
        donate = tuple(range(n_params, n_params + len(out_names)))

        def _body(*args):
            operands = list(args)
            if pname is not None:
                operands.append(b2j.partition_id_tensor())
            outs = b2j._bass_exec_p.bind(
                *operands, out_avals=tuple(out_avals),
                in_names=tuple(all_names),
                out_names=tuple(out_names), lowering_input_output_aliases=(),
                sim_require_finite=True, sim_require_nnan=True, nc=nc)
            return tuple(outs)

        mesh = Mesh(np.asarray(jax.devices()[:8]), ("core",))
        nio = n_params + len(out_names)
        sharded = jax.jit(
            shard_map(_body, mesh=mesh,
                      in_specs=(PartitionSpec("core"),) * nio,
                      out_specs=(PartitionSpec("core"),) * len(out_names),
                      check_rep=False),
            donate_argnums=donate, keep_unused=True)
        from jax.sharding import NamedSharding
        shspec = NamedSharding(mesh, PartitionSpec("core"))
        _CACHE["exec"] = (sharded, in_names, out_names, out_avals,
                         zero_shapes, shspec)
        return res.results

    sharded, in_names, out_names, out_avals, zero_shapes, _ = _CACHE["exec"]
    concat_in = [np.concatenate([m[name] for m in in_maps], axis=0)
                 for name in in_names]
    concat_zeros = [np.zeros((8 * s[0], *s[1:]), d) for (s, d) in zero_shapes]
    out_arrs = sharded(*concat_in, *concat_zeros)
    return [
        {name: np.asarray(out_arrs[i]).reshape(8, *out_avals[i].shape)[c]
         for i, name in enumerate(out_names)}
        for c in range(8)
    ]


def kernel(**inputs):
    hf = np.asarray(inputs["harmonic_frequencies"], np.float32)
    ha = np.asarray(inputs["harmonic_amplitudes"], np.float32)
    nbaf = np.asarray(inputs["noisebank_amplitudes"], np.float32)
    nbe = np.asarray(inputs["noisebank_mod_exponents"], np.float32)
    nbw = np.asarray(inputs["noisebank_mod_weights"], np.float32)
    pg = np.asarray(inputs["pulse_noise_gain"], np.float32)
    fg = np.asarray(inputs["flow_noise_gain"], np.float32)
    ip = np.asarray(inputs["initial_phase"], np.float32)
    nbands = np.asarray(inputs["noise_bands"], np.float32)

    if "nc" not in _CACHE:
        _CACHE["nc"] = _build()
    nc = _CACHE["nc"]

    # quantize (all fp32-path, no float64 temporaries on the big arrays);
    # numpy releases the GIL on large ufuncs, so run the three big
    # conversions in parallel.
    from concurrent.futures import ThreadPoolExecutor

    q8 = np.empty((B, H, T), np.uint8)
    excl = np.empty((B, H, NTB * TS // TC), np.float64)

    def _quant_freq(b, r0, r1):
        # error-feedback quantization: q = diff(round(cumsum(f*1024/48000)))
        S = np.cumsum(hf[b, r0:r1].astype(np.float64) * (MODQ / SR), axis=-1)
        np.round(S, out=S)
        qv = q8[b, r0:r1]
        qv[:, 0] = S[:, 0]
        np.subtract(S[:, 1:], S[:, :-1], out=qv[:, 1:], casting="unsafe")
        # exact carries: phase before chunk c is R at the previous chunk end
        Rc = S.reshape(r1 - r0, NTB * TS // TC, TC)[..., -1]
        excl[b, r0:r1, 0] = 0.0
        excl[b, r0:r1, 1:] = Rc[..., :-1] % MODQ

    # On cached calls, upload each input group to the devices as soon as its
    # quantization finishes (jax.device_put with the mesh sharding in worker
    # threads) so the host quant hides behind the ~35 MB/s tunnel.
    fast = "exec" in _CACHE
    puts = {}
    zero_futs = []
    if "pool" not in _CACHE:
        _CACHE["pool"] = ThreadPoolExecutor(24)
    ex = _CACHE["pool"]
    amp8 = np.empty((B, H, T), np.uint8)

    def _quant_amp(b):
        amp8[b] = (ha[b] * np.float32(255.0)
                   + np.float32(0.5)).astype(np.uint8)

    if True:
        fqs = [ex.submit(_quant_freq, b, r0, r0 + 64)
               for b in range(B) for r0 in (0, 64)]
        fas = [ex.submit(_quant_amp, b) for b in range(B)]
        fn = ex.submit(lambda: ((nbaf * np.float32(255.0)
                                 + np.float32(0.5)).astype(np.uint8),
                                nbands.astype(ml_dtypes.bfloat16)))
        if fast:
            import jax
            (sharded, in_names, out_names, out_avals, zero_shapes,
             shspec) = _CACHE["exec"]

            def up_cat(parts):
                return jax.device_put(np.concatenate(parts, axis=0), shspec)

            for (s, d) in zero_shapes:
                zero_futs.append(ex.submit(
                    lambda s=s, d=d: jax.device_put(
                        np.zeros((8 * s[0], *s[1:]), d), shspec)))
        # amp/noise quant finishes long before the float64 freq cumsum:
        # get their uploads onto the wire first so it stays busy
        for f in fas:
            f.result()
        if fast:
            for k in range(8):
                puts[f"amp{k}"] = ex.submit(up_cat, [
                    amp8[c // 2,
                         (c % 2) * 64 + k * 8:(c % 2) * 64 + (k + 1) * 8]
                    for c in range(8)])
        nba8, bandsbf = fn.result()
        if fast:
            for k in range(4):
                puts[f"nba{k}"] = ex.submit(up_cat, [
                    nba8[c // 2,
                         (c % 2) * 16 + k * 4:(c % 2) * 16 + (k + 1) * 4]
                    for c in range(8)])
                puts[f"nbb{k}"] = ex.submit(up_cat, [
                    bandsbf[(c % 2) * 16 + k * 4:(c % 2) * 16 + (k + 1) * 4]
                    for c in range(8)])
        for f in fqs:
            f.result()
        if fast:
            for k in range(8):
                puts[f"q{k}"] = ex.submit(up_cat, [
                    q8[c // 2, (c % 2) * 64 + k * 8:(c % 2) * 64 + (k + 1) * 8]
                    for c in range(8)])

    phi0q = ((ip[..., 0].astype(np.float64) + np.pi / 2)
             * (MODQ / (2.0 * np.pi)))                      # [B,H]
    vals = (excl + phi0q[:, :, None]) % MODQ                # [B,H,64]

    p = np.arange(128)
    tbp = p % 8
    lhsT = np.zeros((128, 64), np.float32)
    for jj in range(8):
        sel = tbp == jj
        lhsT[sel, jj] = -1.0 / 256         # hc (sign undoes the -sin fold)
        lhsT[sel, 32 + jj] = 1.0 / 256     # nz
    lhsT = lhsT.astype(ml_dtypes.bfloat16)
    m_p = (p % 32) // 8
    cl_p = p // 32
    jj32 = np.arange(32)
    ind_mod = ((cl_p[:, None] == jj32[None, :] // 8) &
               (tbp[:, None] == jj32[None, :] % 8)).astype(np.float32)

    smalls = {"lhsT": [], "wlhsT": [], "ecol": [], "init": []}
    in_maps = []
    for core in range(8):
        b, j = divmod(core, 2)
        h0 = j * 64
        ns0 = j * 16
        vb = vals[b, h0:h0 + 64]            # [64 h_local, 64 chunk-ordinal]
        init = np.empty((128, 32), np.float32)
        for g in range(NG):
            init[:, g * 8:(g + 1) * 8] = \
                vb[g * HG:(g + 1) * HG].reshape(128, 8)
        wl = (ind_mod * nbw[b, m_p, 0][:, None]).astype(ml_dtypes.bfloat16)
        ec = nbe[b, m_p, 0].astype(np.float32).reshape(128, 1)
        smalls["init"].append(init)
        smalls["lhsT"].append(lhsT)
        smalls["wlhsT"].append(wl)
        smalls["ecol"].append(ec)
        if not fast:
            m = dict(init=init, lhsT=lhsT, wlhsT=wl, ecol=ec)
            for k in range(8):
                rs = slice(h0 + k * 8, h0 + (k + 1) * 8)
                m[f"q{k}"] = q8[b, rs]
                m[f"amp{k}"] = amp8[b, rs]
            for k in range(4):
                m[f"nba{k}"] = nba8[b, ns0 + k * 4:ns0 + (k + 1) * 4]
                m[f"nbb{k}"] = bandsbf[ns0 + k * 4:ns0 + (k + 1) * 4]
            in_maps.append(m)

    if fast:
        args = [puts[n].result() if n in puts
                else np.concatenate(smalls[n], axis=0) for n in in_names]
        out_arrs = sharded(*args, *[zf.result() for zf in zero_futs])
        fetches = [ex.submit(np.asarray, a) for a in out_arrs]
        fulls = [f.result().reshape(8, *out_avals[i].shape)
                 for i, f in enumerate(fetches)]
        outs = [{name: fulls[i][c] for i, name in enumerate(out_names)}
                for c in range(8)]
    else:
        outs = _run_cores(nc, in_maps)

    sc_hc = np.float32(256.0 / 255.0)
    sc_nz = np.float32(256.0 / 255.0)
    out = np.empty((B, 1, T), np.float32)
    for b in range(B):
        r0 = outs[2 * b]
        o0 = r0["out"].astype(np.float32)
        o1 = outs[2 * b + 1]["out"].astype(np.float32)
        hc = (o0[0:8].reshape(T) + o1[0:8].reshape(T)) * sc_hc
        noise = (o0[8:16].reshape(T) + o1[8:16].reshape(T)) * sc_nz
        # md[half, j', tl]: j' = cl*8 + tb; t = tb*8192 + (half*4+cl)*1024 + tl
        md = r0["md_out"].astype(np.float32).reshape(2, 4, 8, TC)
        msum = np.ascontiguousarray(md.transpose(2, 0, 1, 3)).reshape(T)
        pgb = pg[b, 0, 0]
        fgb = fg[b, 0, 0]
        tg = (pgb + fgb) * np.float32(0.7)
        out[b, 0] = (hc + msum * noise * pgb + hc * noise * tg
                     + noise * fgb * np.float32(0.3))
    return out


# revision 71
# speedup vs baseline: 1.0055x; 1.0055x over previous
"""HarmonicNoiseSynth Trainium2 kernel (v2).

Sharding: 8 cores = 4 batches x 2 harmonic halves (64 harmonics each); each
core also handles 16 of the 32 noise bands; every core runs the modulator
path on its first 4 local harmonics but the host only consumes it from j==0
cores (where those are the true modulators, harmonics 0..3).

Wire format (host-quantized to cut the ~30 MB/s axon transfer 3x):
  freq  -> uint16 phase units q = round(f * 65536/48000); phase quantization
           errors are independent per sample so the phase error random-walks:
           sigma ~ (2pi/65536)*sqrt(T)*0.29 ~ 5e-3 rad. Negligible.
  amp   -> uint8 (x255), dequant via SWDGE DMA cast to bf16 + 1/256 in lhsT,
           with a 256/255 host-side correction.
  nba   -> uint8 (x255), bands -> bf16.

Device pipeline per [128, 4096] half-slice (partition p = h_local*8 + tb,
tb = time slice of 8192; free dim = time within slice):
  1. scan (DVE): cumsum of q in fp32 (exact: chunk sums < 2^24), int32 out,
     initial = host-computed (phi0 + carry) mod 65536 per 1024-chunk.
  2. AND 0xFFFF (DVE): phase mod 65536 (per-element range reduction).
  3. Sin (ACT): cos via Sin(2pi/65536 * m - pi) = -cos(theta); the -1 is
     folded into lhsT. bf16 out.
  4. amp mul (POOL, bf16); 5. PE matmul partial sums, accumulated across the
     4 harmonic groups in PSUM ([64, 4096] tile: rows 0-7/8-15 hc/nz half 0,
     rows 32-39/40-47 hc/nz half 1).
The modulator path runs at the end, reusing the PSUM tile after evacuation;
|arcsin(0.99 c)|^e = Exp(e * Ln((2/pi)*Arctan(y*Rsqrt(1-y^2)))), y=Abs(.99c)
with the abs/scale/exponent multiplies folded into ACT affine slots.
"""
import sys

sys.path.insert(0, "/opt/trn_rl_repo")

import numpy as np
import ml_dtypes

import concourse.bass as bass
import concourse.mybir as mybir
from concourse.tile import TileContext
from concourse.bass_utils import run_bass_kernel_spmd

F = mybir.dt.float32
BF = mybir.dt.bfloat16
U8 = mybir.dt.uint8
U16 = mybir.dt.uint16
I32 = mybir.dt.int32

SR = 48000.0
B, H, NB, T = 4, 128, 32, 65536
NTB = 8          # time slices on partitions
TS = T // NTB    # 8192 per slice
TC = 1024        # scan chunk columns (cumsum stays < 2^24: exact fp32)
HB = 4096        # half-slice processed per pipeline step
NG = 4           # h-groups of 16 harmonics
HG = 16
# 9-bit phase units: freq ships as uint8 *increments of the rounded
# cumulative phase* (error-feedback quantization: q[t] = round(S[t]) -
# round(S[t-1]) with S = cumsum(f*MODQ/48000)), so the accumulated phase
# error is bounded by half a quantum (pi/MODQ rad) at every t instead of
# random-walking. M=512 keeps increments <= 88 (6.5-bit entropy — the axon
# tunnel's compressor moves that ~12% faster than the M=1024 stream).
MODQ = 512.0
SCALE_Q = float(MODQ / SR)
KP = float(2.0 * np.pi / MODQ)

_CACHE = {}

# which (h5, g) half-slices run the amp*cos multiply on DVE (rest on Pool)
# and where the noise mul runs: tuned with the TimelineSim cost model.
MUL_DVE_STEPS = {0, 1, 2, 3, 4, 5, 6, 7}
NOISE_MUL_DVE = False


def _split_multiwaits(nc):
    """This walrus build supports ONE sync wait per instruction; hoist extras
    onto single-wait NoOps inserted before the offending instruction."""
    ctr = 0
    for f in nc.m.functions:
        for bb in f.blocks:
            insts = list(bb.instructions)
            if not any(i.sync_info is not None and len(i.sync_info.on_wait) > 1
                       for i in insts):
                continue
            new = []
            for inst in insts:
                si = inst.sync_info
                if si is not None and len(si.on_wait) > 1:
                    waits = list(si.on_wait)
                    for w in waits[:-1]:
                        ctr += 1
                        nop = mybir.InstNoOp(name=f"mwsplit_{ctr}",
                                             engine=inst.engine)
                        nop.sync_info = mybir.SyncInfo(on_wait=[w], on_update=[])
                        new.append(nop)
                    inst.sync_info = mybir.SyncInfo(on_wait=[waits[-1]],
                                                    on_update=list(si.on_update))
                new.append(inst)
            bb.instructions = new
    return ctr


def _build():
    nc = bass.Bass("TRN2")
    HN = H // 2  # 64 harmonics per core

    q_ds = [nc.dram_tensor(f"q{k}", [8, T], U8, kind="ExternalInput")
            for k in range(8)]
    amp_ds = [nc.dram_tensor(f"amp{k}", [8, T], U8, kind="ExternalInput")
              for k in range(8)]
    nba_ds = [nc.dram_tensor(f"nba{k}", [4, T], U8, kind="ExternalInput")
              for k in range(4)]
    nbb_ds = [nc.dram_tensor(f"nbb{k}", [4, T], BF, kind="ExternalInput")
              for k in range(4)]
    init_d = nc.dram_tensor("init", [128, 32], F, kind="ExternalInput")
    lhsT_d = nc.dram_tensor("lhsT", [128, 64], BF, kind="ExternalInput")
    wlhsT_d = nc.dram_tensor("wlhsT", [128, 32], BF, kind="ExternalInput")
    ecol_d = nc.dram_tensor("ecol", [128, 1], F, kind="ExternalInput")

    out_d = nc.dram_tensor("out", [16, TS], BF, kind="ExternalOutput")
    md_d = nc.dram_tensor("md_out", [2, 32, TC], BF, kind="ExternalOutput")

    # [64, 8192] each: tensor k covers harmonics 8k..8k+8 -> partitions
    # (h_local*8 + tb) within its 64-row slab
    q_rs = [d[:, :].rearrange("h (tb t) -> (h tb) t", tb=NTB) for d in q_ds]
    amp_rs = [d[:, :].rearrange("h (tb t) -> (h tb) t", tb=NTB)
              for d in amp_ds]                                     # [64, 8192]
    nba_rs = [d[:, :].rearrange("n (tb t) -> (n tb) t", tb=NTB)
              for d in nba_ds]                                     # [32, 8192]
    nbb_rs = [d[:, :].rearrange("n (tb t) -> (n tb) t", tb=NTB)
              for d in nbb_ds]                                     # [32, 8192]

    with TileContext(nc) as tc:
        with tc.tile_pool(name="sm", bufs=1) as sm, \
             tc.tile_pool(name="st", bufs=2) as st, \
             tc.tile_pool(name="pp", bufs=1, space="PSUM") as pp:

            lhsT = sm.tile([128, 64], BF)
            nc.sync.dma_start(out=lhsT, in_=lhsT_d[:, :])
            wlhsT = sm.tile([128, 32], BF)
            nc.sync.dma_start(out=wlhsT, in_=wlhsT_d[:, :])
            ecol = sm.tile([128, 1], F)
            nc.sync.dma_start(out=ecol, in_=ecol_d[:, :])
            init_sb = sm.tile([128, 32], F)
            nc.sync.dma_start(out=init_sb, in_=init_d[:, :])
            bsin = sm.tile([128, 1], F)
            nc.vector.memset(bsin, -np.pi)
            bone = sm.tile([128, 1], F)
            nc.vector.memset(bone, 1.0)
            stg = [sm.tile([128, TC], BF, tag=f"stg{i}", name=f"stg{i}")
                   for i in range(2)]
            # rows 0-7 hold hc, rows 32-39 hold nz (partition-aligned w/ psum)
            hcnz = [sm.tile([40, HB], BF, tag=f"hcnz{i}", name=f"hcnz{i}")
                    for i in range(2)]

            # PSUM (matmul out base partition must be 0/32/64): rows 0-7 hc,
            # rows 32-39 nz for the current half; halves run sequentially.
            ps = pp.tile([64, HB], F, name="ps")

            for h5 in range(2):
                cols = slice(h5 * HB, (h5 + 1) * HB)
                for g in range(NG):
                    col0 = g * 8 + h5 * 4
                    qt = st.tile([128, HB], U8, tag="qt")
                    for k in range(2):
                        nc.sync.dma_start(
                            out=qt[k * 64:(k + 1) * 64, :],
                            in_=q_rs[g * 2 + k][:, cols])
                    at = st.tile([128, HB], BF, tag="at")
                    for k in range(2):
                        nc.gpsimd.dma_start(
                            out=at[k * 64:(k + 1) * 64, :],
                            in_=amp_rs[g * 2 + k][:, cols])
                    y = st.tile([128, HB], I32, tag="y")
                    for c in range(HB // TC):
                        nc.vector.tensor_tensor_scan(
                            out=y[:, c * TC:(c + 1) * TC],
                            data0=qt[:, c * TC:(c + 1) * TC],
                            data1=qt[:, c * TC:(c + 1) * TC],
                            initial=init_sb[:, col0 + c:col0 + c + 1],
                            op0=mybir.AluOpType.add,
                            op1=mybir.AluOpType.bypass)
                    nc.vector.tensor_scalar(out=y, in0=y,
                                            scalar1=int(MODQ) - 1,
                                            scalar2=None,
                                            op0=mybir.AluOpType.bitwise_and)
                    cosb = st.tile([128, HB], BF, tag="cosb")
                    nc.scalar.activation(out=cosb, in_=y,
                                         func=mybir.ActivationFunctionType.Sin,
                                         scale=KP, bias=bsin)
                    if g == 0:
                        for cl in range(4):
                            nc.sync.dma_start(
                                out=stg[h5][cl * 32:(cl + 1) * 32, :],
                                in_=cosb[0:32, cl * TC:(cl + 1) * TC])
                    if h5 * 4 + g in MUL_DVE_STEPS:
                        nc.vector.tensor_mul(out=cosb, in0=cosb, in1=at)
                    else:
                        nc.gpsimd.tensor_mul(out=cosb, in0=cosb, in1=at)
                    for s in range(HB // 512):
                        nc.tensor.matmul(
                            ps[0:8, s * 512:(s + 1) * 512],
                            lhsT[:, 0:8],
                            cosb[:, s * 512:(s + 1) * 512],
                            start=(g == 0), stop=(g == NG - 1))

                # noise: 16 bands x 8 tb on partitions
                bt = st.tile([128, HB], BF, tag="bt", bufs=1)
                for k in range(4):
                    nc.sync.dma_start(out=bt[k * 32:(k + 1) * 32, :],
                                      in_=nbb_rs[k][:, cols])
                an = st.tile([128, HB], BF, tag="an", bufs=1)
                for k in range(4):
                    nc.gpsimd.dma_start(out=an[k * 32:(k + 1) * 32, :],
                                        in_=nba_rs[k][:, cols])
                if NOISE_MUL_DVE:
                    nc.vector.tensor_mul(out=bt, in0=bt, in1=an)
                else:
                    nc.gpsimd.tensor_mul(out=bt, in0=bt, in1=an)
                for s in range(HB // 512):
                    nc.tensor.matmul(ps[32:40, s * 512:(s + 1) * 512],
                                     lhsT[:, 32:40],
                                     bt[:, s * 512:(s + 1) * 512],
                                     start=True, stop=True)

                # evacuate hc (psum rows 0-7) + nz (rows 32-39)
                nc.scalar.copy(out=hcnz[h5][0:8, :], in_=ps[0:8, :])
                nc.scalar.copy(out=hcnz[h5][32:40, :], in_=ps[32:40, :])
                nc.sync.dma_start(out=out_d[0:8, h5 * HB:(h5 + 1) * HB],
                                  in_=hcnz[h5][0:8, :])
                nc.sync.dma_start(out=out_d[8:16, h5 * HB:(h5 + 1) * HB],
                                  in_=hcnz[h5][32:40, :])

            # ---- modulator path (staged -cos of local harmonics 0..3) ----
            ys, y2s = [], []
            for h5 in range(2):
                yv = sm.tile([128, TC], F, tag=f"my{h5}", name=f"my{h5}")
                nc.scalar.activation(out=yv, in_=stg[h5],
                                     func=mybir.ActivationFunctionType.Abs,
                                     scale=0.99)
                ys.append(yv)
            for h5 in range(2):
                y2 = sm.tile([128, TC], F, tag=f"my2{h5}", name=f"my2{h5}")
                nc.vector.tensor_mul(out=y2, in0=ys[h5], in1=ys[h5])
                y2s.append(y2)
            for h5 in range(2):   # s = sqrt(1 - y^2)
                nc.scalar.activation(out=y2s[h5], in_=y2s[h5],
                                     func=mybir.ActivationFunctionType.Sqrt,
                                     scale=-1.0, bias=bone)
            for h5 in range(2):   # r = 1/s
                nc.vector.reciprocal(out=y2s[h5], in_=y2s[h5])
            for h5 in range(2):   # t = y * r  (= tan(arcsin y))
                nc.vector.tensor_mul(out=ys[h5], in0=ys[h5], in1=y2s[h5])
            for h5 in range(2):
                nc.scalar.activation(out=ys[h5], in_=ys[h5],
                                     func=mybir.ActivationFunctionType.Arctan)
            for h5 in range(2):   # l = ln((2/pi) * arctan)
                nc.scalar.activation(out=ys[h5], in_=ys[h5],
                                     func=mybir.ActivationFunctionType.Ln,
                                     scale=float(2.0 / np.pi))
            shp = []
            for h5 in range(2):   # shaped = exp(e * l), bf16 for the matmul
                sb = sm.tile([128, TC], BF, tag=f"msh{h5}", name=f"msh{h5}")
                nc.scalar.activation(out=sb, in_=ys[h5],
                                     func=mybir.ActivationFunctionType.Exp,
                                     scale=ecol)
                shp.append(sb)
            for h5 in range(2):   # reuse evacuated psum banks for md
                mps = ps[0:32, h5 * TC:(h5 + 1) * TC]
                for s in range(TC // 512):
                    nc.tensor.matmul(mps[:, s * 512:(s + 1) * 512], wlhsT,
                                     shp[h5][:, s * 512:(s + 1) * 512],
                                     start=True, stop=True)
                mcp = sm.tile([32, TC], BF, tag=f"mcp{h5}", name=f"mcp{h5}")
                nc.scalar.copy(out=mcp, in_=mps)
                nc.sync.dma_start(out=md_d[h5, :, :], in_=mcp)

    _split_multiwaits(nc)
    return nc


def _run_cores(nc, in_maps):
    """First call: canonical run_bass_kernel_spmd (compiles the NEFF via the
    neuronx hook). Later calls: a cached jit of the same bass2jax executable —
    rebuilding the jit per call re-traces and re-lowers the Bass module each
    time, which costs seconds."""
    if "exec" not in _CACHE:
        res = run_bass_kernel_spmd(nc, in_maps, core_ids=list(range(8)))
        import jax
        import concourse.bass2jax as b2j
        import concourse.mybir as mb
        from jax.sharding import Mesh, PartitionSpec
        from jax.experimental.shard_map import shard_map

        pname = (nc.partition_id_tensor.name if nc.partition_id_tensor
                 else None)
        in_names, out_names, out_avals, zero_shapes = [], [], [], []
        for alloc in nc.m.functions[0].allocations:
            if not isinstance(alloc, mb.MemoryLocationSet):
                continue
            name = alloc.memorylocations[0].name
            if alloc.kind == "ExternalInput":
                if name != pname:
                    in_names.append(name)
            elif alloc.kind == "ExternalOutput":
                out_names.append(name)
                shape = tuple(alloc.tensor_shape)
                dtype = mb.dt.np(alloc.dtype)
                out_avals.append(jax.core.ShapedArray(shape, dtype))
                zero_shapes.append((shape, dtype))
        n_params = len(in_names)
        all_names = in_names + out_names
        if pname is not None:
            all_names = all_names + [pname]
        donate = tuple(range(n_params, n_params + len(out_names)))

        def _body(*args):
            operands = list(args)
            if pname is not None:
                operands.append(b2j.partition_id_tensor())
            outs = b2j._bass_exec_p.bind(
                *operands, out_avals=tuple(out_avals),
                in_names=tuple(all_names),
                out_names=tuple(out_names), lowering_input_output_aliases=(),
                sim_require_finite=True, sim_require_nnan=True, nc=nc)
            return tuple(outs)

        mesh = Mesh(np.asarray(jax.devices()[:8]), ("core",))
        nio = n_params + len(out_names)
        sharded = jax.jit(
            shard_map(_body, mesh=mesh,
                      in_specs=(PartitionSpec("core"),) * nio,
                      out_specs=(PartitionSpec("core"),) * len(out_names),
                      check_rep=False),
            donate_argnums=donate, keep_unused=True)
        from jax.sharding import NamedSharding
        shspec = NamedSharding(mesh, PartitionSpec("core"))
        _CACHE["exec"] = (sharded, in_names, out_names, out_avals,
                         zero_shapes, shspec)
        return res.results

    sharded, in_names, out_names, out_avals, zero_shapes, _ = _CACHE["exec"]
    concat_in = [np.concatenate([m[name] for m in in_maps], axis=0)
                 for name in in_names]
    concat_zeros = [np.zeros((8 * s[0], *s[1:]), d) for (s, d) in zero_shapes]
    out_arrs = sharded(*concat_in, *concat_zeros)
    return [
        {name: np.asarray(out_arrs[i]).reshape(8, *out_avals[i].shape)[c]
         for i, name in enumerate(out_names)}
        for c in range(8)
    ]


def kernel(**inputs):
    hf = np.asarray(inputs["harmonic_frequencies"], np.float32)
    ha = np.asarray(inputs["harmonic_amplitudes"], np.float32)
    nbaf = np.asarray(inputs["noisebank_amplitudes"], np.float32)
    nbe = np.asarray(inputs["noisebank_mod_exponents"], np.float32)
    nbw = np.asarray(inputs["noisebank_mod_weights"], np.float32)
    pg = np.asarray(inputs["pulse_noise_gain"], np.float32)
    fg = np.asarray(inputs["flow_noise_gain"], np.float32)
    ip = np.asarray(inputs["initial_phase"], np.float32)
    nbands = np.asarray(inputs["noise_bands"], np.float32)

    if "nc" not in _CACHE:
        _CACHE["nc"] = _build()
    nc = _CACHE["nc"]

    # quantize (all fp32-path, no float64 temporaries on the big arrays);
    # numpy releases the GIL on large ufuncs, so run the three big
    # conversions in parallel.
    from concurrent.futures import ThreadPoolExecutor

    q8 = np.empty((B, H, T), np.uint8)
    excl = np.empty((B, H, NTB * TS // TC), np.float64)

    def _quant_freq(b, r0, r1):
        # error-feedback quantization: q = diff(round(cumsum(f*1024/48000)))
        S = np.cumsum(hf[b, r0:r1].astype(np.float64) * (MODQ / SR), axis=-1)
        np.round(S, out=S)
        qv = q8[b, r0:r1]
        qv[:, 0] = S[:, 0]
        np.subtract(S[:, 1:], S[:, :-1], out=qv[:, 1:], casting="unsafe")
        # exact carries: phase before chunk c is R at the previous chunk end
        Rc = S.reshape(r1 - r0, NTB * TS // TC, TC)[..., -1]
        excl[b, r0:r1, 0] = 0.0
        excl[b, r0:r1, 1:] = Rc[..., :-1] % MODQ

    # On cached calls, upload each input group to the devices as soon as its
    # quantization finishes (jax.device_put with the mesh sharding in worker
    # threads) so the host quant hides behind the ~35 MB/s tunnel.
    fast = "exec" in _CACHE
    puts = {}
    zero_futs = []
    if "pool" not in _CACHE:
        _CACHE["pool"] = ThreadPoolExecutor(24)
    ex = _CACHE["pool"]
    amp8 = np.empty((B, H, T), np.uint8)

    def _quant_amp(b):
        amp8[b] = (ha[b] * np.float32(255.0)
                   + np.float32(0.5)).astype(np.uint8)

    if True:
        fqs = [ex.submit(_quant_freq, b, r0, r0 + 64)
               for b in range(B) for r0 in (0, 64)]
        fas = [ex.submit(_quant_amp, b) for b in range(B)]
        fn = ex.submit(lambda: ((nbaf * np.float32(255.0)
                                 + np.float32(0.5)).astype(np.uint8),
                                nbands.astype(ml_dtypes.bfloat16)))
        if fast:
            import jax
            (sharded, in_names, out_names, out_avals, zero_shapes,
             shspec) = _CACHE["exec"]

            def up_cat(parts):
                return jax.device_put(np.concatenate(parts, axis=0), shspec)

            for (s, d) in zero_shapes:
                zero_futs.append(ex.submit(
                    lambda s=s, d=d: jax.device_put(
                        np.zeros((8 * s[0], *s[1:]), d), shspec)))
        # amp/noise quant finishes long before the float64 freq cumsum:
        # get their uploads onto the wire first so it stays busy
        for f in fas:
            f.result()
        if fast:
            for k in range(8):
                puts[f"amp{k}"] = ex.submit(up_cat, [
                    amp8[c // 2,
                         (c % 2) * 64 + k * 8:(c % 2) * 64 + (k + 1) * 8]
                    for c in range(8)])
        nba8, bandsbf = fn.result()
        if fast:
            for k in range(4):
                puts[f"nba{k}"] = ex.submit(up_cat, [
                    nba8[c // 2,
                         (c % 2) * 16 + k * 4:(c % 2) * 16 + (k + 1) * 4]
                    for c in range(8)])
                puts[f"nbb{k}"] = ex.submit(up_cat, [
                    bandsbf[(c % 2) * 16 + k * 4:(c % 2) * 16 + (k + 1) * 4]
                    for c in range(8)])
        for f in fqs:
            f.result()
        if fast:
            for k in range(8):
                puts[f"q{k}"] = ex.submit(up_cat, [
                    q8[c // 2, (c % 2) * 64 + k * 8:(c % 2) * 64 + (k + 1) * 8]
                    for c in range(8)])

    phi0q = ((ip[..., 0].astype(np.float64) + np.pi / 2)
             * (MODQ / (2.0 * np.pi)))                      # [B,H]
    vals = (excl + phi0q[:, :, None]) % MODQ                # [B,H,64]

    p = np.arange(128)
    tbp = p % 8
    lhsT = np.zeros((128, 64), np.float32)
    for jj in range(8):
        sel = tbp == jj
        lhsT[sel, jj] = -1.0 / 256         # hc (sign undoes the -sin fold)
        lhsT[sel, 32 + jj] = 1.0 / 256     # nz
    lhsT = lhsT.astype(ml_dtypes.bfloat16)
    m_p = (p % 32) // 8
    cl_p = p // 32
    jj32 = np.arange(32)
    ind_mod = ((cl_p[:, None] == jj32[None, :] // 8) &
               (tbp[:, None] == jj32[None, :] % 8)).astype(np.float32)

    smalls = {"lhsT": [], "wlhsT": [], "ecol": [], "init": []}
    in_maps = []
    for core in range(8):
        b, j = divmod(core, 2)
        h0 = j * 64
        ns0 = j * 16
        vb = vals[b, h0:h0 + 64]            # [64 h_local, 64 chunk-ordinal]
        init = np.empty((128, 32), np.float32)
        for g in range(NG):
            init[:, g * 8:(g + 1) * 8] = \
                vb[g * HG:(g + 1) * HG].reshape(128, 8)
        wl = (ind_mod * nbw[b, m_p, 0][:, None]).astype(ml_dtypes.bfloat16)
        ec = nbe[b, m_p, 0].astype(np.float32).reshape(128, 1)
        smalls["init"].append(init)
        smalls["lhsT"].append(lhsT)
        smalls["wlhsT"].append(wl)
        smalls["ecol"].append(ec)
        if not fast:
            m = dict(init=init, lhsT=lhsT, wlhsT=wl, ecol=ec)
            for k in range(8):
                rs = slice(h0 + k * 8, h0 + (k + 1) * 8)
                m[f"q{k}"] = q8[b, rs]
                m[f"amp{k}"] = amp8[b, rs]
            for k in range(4):
                m[f"nba{k}"] = nba8[b, ns0 + k * 4:ns0 + (k + 1) * 4]
                m[f"nbb{k}"] = bandsbf[ns0 + k * 4:ns0 + (k + 1) * 4]
            in_maps.append(m)

    if fast:
        args = [puts[n].result() if n in puts
                else np.concatenate(smalls[n], axis=0) for n in in_names]
        out_arrs = sharded(*args, *[zf.result() for zf in zero_futs])
        fetches = [ex.submit(np.asarray, a) for a in out_arrs]
        fulls = [f.result().reshape(8, *out_avals[i].shape)
                 for i, f in enumerate(fetches)]
        outs = [{name: fulls[i][c] for i, name in enumerate(out_names)}
                for c in range(8)]
    else:
        outs = _run_cores(nc, in_maps)

    sc_hc = np.float32(256.0 / 255.0)
    sc_nz = np.float32(256.0 / 255.0)
    out = np.empty((B, 1, T), np.float32)
    for b in range(B):
        r0 = outs[2 * b]
        o0 = r0["out"].astype(np.float32)
        o1 = outs[2 * b + 1]["out"].astype(np.float32)
        hc = (o0[0:8].reshape(T) + o1[0:8].reshape(T)) * sc_hc
        noise = (o0[8:16].reshape(T) + o1[8:16].reshape(T)) * sc_nz
        # md[half, j', tl]: j' = cl*8 + tb; t = tb*8192 + (half*4+cl)*1024 + tl
        md = r0["md_out"].astype(np.float32).reshape(2, 4, 8, TC)
        msum = np.ascontiguousarray(md.transpose(2, 0, 1, 3)).reshape(T)
        pgb = pg[b, 0, 0]
        fgb = fg[b, 0, 0]
        tg = (pgb + fgb) * np.float32(0.7)
        out[b, 0] = (hc + msum * noise * pgb + hc * noise * tg
                     + noise * fgb * np.float32(0.3))
    return out
